# revision 1
# baseline (speedup 1.0000x reference)
"""RoPE + ALiBi single-head attention (B=8, T=2048, H=256) on 8 Trainium2
cores, batch-parallel (one batch element per core).

Per-core algorithm (all compute on device):
  qeT/keT = RoPE(qT/kT)                     [DVE, fp32 -> fp32r, pipelined
                                             with the input DMA in 512-col
                                             chunks so GEMM1 starts early]
  scoresT[s,t] = sum_d keT[d,s]*qeT[d,t]    [PE, fp32r, 2 k-tiles]
  at[s,t] = exp(scoresT*scale + slope*s)    [ACT, PSUM->SBUF fp32r]
     (the -slope*t alibi term is constant per softmax column and cancels)
  den[t] = sum_s at[s,t]                    [PE: 16 accumulating ones-matmuls
                                             into a [1,512] PSUM row]
  outT[h,t] = (sum_s v[s,h]*at[s,t]) / den  [PE fp32r; reciprocal via magic
                                             bit-trick + 3 Newton steps on
                                             the [1,512] row, broadcast on
                                             GpSimd, DVE normalize]
Host only reshapes/transposes and precomputes the rope/alibi tables.
"""
import math
from contextlib import ExitStack

import numpy as np

import concourse.bacc as bacc
import concourse.tile as tile
from concourse import mybir
from concourse.bass_utils import run_bass_kernel_spmd

B, T, H = 8, 2048, 256
HALF = H // 2          # 128 (rope half, also partition dim)
NCHUNK = 4
CHUNK = T // NCHUNK    # 512 query columns per chunk
NS = T // 128          # 16 key tiles
ROPE_BASE = 10000.0
SLOPE = 2.0 ** (-8.0)
SCALE = 1.0 / math.sqrt(H)
RECIP_MAGIC = 0x7EF127EA  # fast fp32 reciprocal seed: magic - bits(x)

F32 = mybir.dt.float32
F32R = mybir.dt.float32r
I32 = mybir.dt.int32
EXP = mybir.ActivationFunctionType.Exp
MULT = mybir.AluOpType.mult
ADD = mybir.AluOpType.add

TRACE = False           # test harness sets True for NTFF profiling
LAST_RESULTS = None     # BassKernelResults of the last run (for profiling)

_NC_CACHE = {}


def _build_nc():
    nc = bacc.Bacc("TRN2", target_bir_lowering=False, debug=False)
    qt_d = nc.dram_tensor("qt", [H, T], F32, kind="ExternalInput").ap()
    kt_d = nc.dram_tensor("kt", [H, T], F32, kind="ExternalInput").ap()
    v_d = nc.dram_tensor("v", [T, H], F32, kind="ExternalInput").ap()
    cos_d = nc.dram_tensor("costab", [HALF, T], F32, kind="ExternalInput").ap()
    sin_d = nc.dram_tensor("sintab", [HALF, T], F32, kind="ExternalInput").ap()
    bias_d = nc.dram_tensor("alibi", [128, NS], F32, kind="ExternalInput").ap()
    ot_d = nc.dram_tensor("ot", [H, T], F32, kind="ExternalOutput").ap()

    with tile.TileContext(nc) as tc, ExitStack() as ctx:
        const = ctx.enter_context(tc.tile_pool(name="const", bufs=1))
        rpool = ctx.enter_context(tc.tile_pool(name="ropeout", bufs=1))
        vpool = ctx.enter_context(tc.tile_pool(name="vpool", bufs=1))
        stage = ctx.enter_context(tc.tile_pool(name="stage", bufs=1))
        atp = ctx.enter_context(tc.tile_pool(name="atp", bufs=26))
        dn = ctx.enter_context(tc.tile_pool(name="dn", bufs=2))
        onp = ctx.enter_context(tc.tile_pool(name="onp", bufs=4))
        ps1p = ctx.enter_context(tc.tile_pool(name="ps1", bufs=3, space="PSUM"))
        ps2p = ctx.enter_context(tc.tile_pool(name="ps2", bufs=3, space="PSUM"))
        pdnp = ctx.enter_context(tc.tile_pool(name="pdn", bufs=2, space="PSUM"))

        # small constants: alibi bias (gpsimd queue), ones column for the
        # denominator partition-reduce matmuls, reciprocal magic row
        biasb = const.tile([128, NS], F32)
        nc.gpsimd.dma_start(biasb[:], bias_d[:])
        ones_f = const.tile([128, 1], F32)
        nc.vector.memset(ones_f[:], 1.0)
        ones_r = const.tile([128, 1], F32R)
        nc.vector.tensor_copy(ones_r[:], ones_f[:])
        magicb = const.tile([1, CHUNK], I32)
        nc.vector.memset(magicb[:], RECIP_MAGIC)

        # persistent fp32r operands for the two GEMMs
        qe = [rpool.tile([128, T], F32R, name=f"qe{i}", tag=f"qe{i}")
              for i in range(2)]
        ke = [rpool.tile([128, T], F32R, name=f"ke{i}", tag=f"ke{i}")
              for i in range(2)]
        vr = vpool.tile([128, NS * H], F32R)

        # full-width staging tiles, filled by per-chunk DMAs (subtile deps
        # let rope/GEMM1 start as soon as their columns land)
        cosb = stage.tile([128, T], F32, tag="cosb")
        sinb = stage.tile([128, T], F32, tag="sinb")
        ks0 = stage.tile([128, T], F32, tag="ks0")
        ks1 = stage.tile([128, T], F32, tag="ks1")
        qs0 = stage.tile([128, T], F32, tag="qs0")
        qs1 = stage.tile([128, T], F32, tag="qs1")

        def load_cols(cc):
            col = slice(cc * CHUNK, (cc + 1) * CHUNK)
            for dst, src in ((cosb, cos_d), (sinb, sin_d),
                             (ks0, kt_d[0:128, :]), (ks1, kt_d[128:256, :])):
                nc.sync.dma_start(dst[:, col], src[:, col])

        def load_q_cols(cc):
            col = slice(cc * CHUNK, (cc + 1) * CHUNK)
            nc.sync.dma_start(qs0[:, col], qt_d[0:128, col])
            nc.sync.dma_start(qs1[:, col], qt_d[128:256, col])

        def rope(src0, src1, dst, col, tmptag):
            """dst0[:,col] = s0*cos - s1*sin ; dst1[:,col] = s1*cos + s0*sin"""
            n = col.stop - col.start
            nc.vector.tensor_mul(dst[0][:, col], src0[:, col], cosb[:, col])
            tmp = stage.tile([128, n], F32, tag="rtmp", bufs=3,
                             name=f"tmp{tmptag}{col.start}")
            nc.vector.tensor_mul(tmp[:], src1[:, col], sinb[:, col])
            nc.vector.tensor_sub(dst[0][:, col], dst[0][:, col], tmp[:])
            nc.vector.tensor_mul(dst[1][:, col], src1[:, col], cosb[:, col])
            tmp2 = stage.tile([128, n], F32, tag="rtmp", bufs=3,
                              name=f"tmp2{tmptag}{col.start}")
            nc.vector.tensor_mul(tmp2[:], src0[:, col], sinb[:, col])
            nc.vector.tensor_add(dst[1][:, col], dst[1][:, col], tmp2[:])

        # chunk-0 inputs first, then k/q rope pipelined with remaining DMAs
        load_cols(0)
        load_q_cols(0)
        rope(ks0, ks1, ke, slice(0, CHUNK), "k0")
        rope(qs0, qs1, qe, slice(0, CHUNK), "q0")
        for cc in range(1, NCHUNK):
            load_cols(cc)
            load_q_cols(cc)
            rope(ks0, ks1, ke, slice(cc * CHUNK, (cc + 1) * CHUNK), f"k{cc}")

        # v load + fp32r cast entirely on gpsimd (own DMA queues, own ALU)
        for s in range(NS):
            vst = stage.tile([128, H], F32, tag="vst", bufs=4, name=f"vst{s}")
            nc.gpsimd.dma_start(vst[:], v_d[s * 128:(s + 1) * 128, :])
            nc.gpsimd.tensor_copy(vr[:, s * H:(s + 1) * H], vst[:])

        mm = nc.tensor.matmul
        for c in range(NCHUNK):
            tcol = slice(c * CHUNK, (c + 1) * CHUNK)
            if c + 1 < NCHUNK:
                # rope next chunk's q columns ahead of its GEMM1
                rope(qs0, qs1, qe, slice((c + 1) * CHUNK, (c + 2) * CHUNK),
                     f"q{c + 1}")
            at_tiles = []
            pden = pdnp.tile([1, CHUNK], F32)
            for s in range(NS):
                p1 = ps1p.tile([128, CHUNK], F32)
                mm(p1[:], ke[0][:, s * 128:(s + 1) * 128], qe[0][:, tcol],
                   start=True, stop=False)
                mm(p1[:], ke[1][:, s * 128:(s + 1) * 128], qe[1][:, tcol],
                   start=False, stop=True)
                if s > 0:
                    # denominator ones-matmul, one tile behind the exps so
                    # the PE never waits on the ACT stream
                    mm(pden[:], ones_r[:, 0:1], at_tiles[s - 1][:],
                       start=(s == 1), stop=False)
                at = atp.tile([128, CHUNK], F32R, tag="at")
                nc.scalar.activation(at[:], p1[:], EXP,
                                     bias=biasb[:, s:s + 1], scale=SCALE)
                at_tiles.append(at)
            mm(pden[:], ones_r[:, 0:1], at_tiles[NS - 1][:],
               start=False, stop=True)

            # reciprocal of the [1, CHUNK] denominator row:
            # seed r = bits(magic - bits(d)), then 3 Newton steps
            den_sb = dn.tile([1, CHUNK], F32, tag="den_sb")
            nc.vector.tensor_copy(den_sb[:], pden[0:1, :])
            r = dn.tile([1, CHUNK], F32, tag="rA", name=f"rA{c}")
            nc.vector.tensor_sub(r[:].bitcast(I32), magicb[:],
                                 den_sb[:].bitcast(I32))
            for it in range(2):
                t2 = dn.tile([1, CHUNK], F32, tag="nt", bufs=2,
                             name=f"nt{c}_{it}")
                nc.vector.scalar_tensor_tensor(t2[:], den_sb[:], -1.0, r[:],
                                               MULT, MULT)
                r_new = dn.tile([1, CHUNK], F32, tag=f"r{it % 2}", bufs=2,
                                name=f"r{c}_{it}")
                nc.vector.scalar_tensor_tensor(r_new[:], t2[:], 2.0, r[:],
                                               ADD, MULT)
                r = r_new
            recipb = dn.tile([128, CHUNK], F32, tag="recipb")
            nc.gpsimd.partition_broadcast(recipb[:], r[0:1, :], 128)

            for h in range(2):
                p2 = ps2p.tile([128, CHUNK], F32)
                for s in range(NS):
                    mm(p2[:], vr[:, s * H + h * 128: s * H + h * 128 + 128],
                       at_tiles[s][:], start=(s == 0), stop=(s == NS - 1))
                on = onp.tile([128, CHUNK], F32)
                nc.vector.tensor_mul(on[:], p2[:], recipb[:])
                nc.sync.dma_start(ot_d[h * 128:(h + 1) * 128, tcol], on[:])

    nc.compile()
    return nc


def _get_nc():
    if "nc" not in _NC_CACHE:
        _NC_CACHE["nc"] = _build_nc()
    return _NC_CACHE["nc"]


def _tables():
    j = np.arange(HALF, dtype=np.float64)
    inv = ROPE_BASE ** (-2.0 * j / H)
    t = np.arange(T, dtype=np.float64)
    fr = np.outer(inv, t)                       # [128, T]
    cos = np.cos(fr).astype(np.float32)
    sin = np.sin(fr).astype(np.float32)
    p = np.arange(128, dtype=np.float64)[:, None]
    sidx = p + 128.0 * np.arange(NS, dtype=np.float64)[None, :]
    bias = (SLOPE * sidx).astype(np.float32)    # [128, NS]
    return cos, sin, bias


def kernel(q, k, v):
    global LAST_RESULTS
    q = np.asarray(q, dtype=np.float32)
    k = np.asarray(k, dtype=np.float32)
    v = np.asarray(v, dtype=np.float32)
    assert q.shape == (B, T, H), q.shape

    nc = _get_nc()
    cos, sin, bias = _tables()
    in_maps = []
    for b in range(B):
        in_maps.append({
            "qt": np.ascontiguousarray(q[b].T),
            "kt": np.ascontiguousarray(k[b].T),
            "v": np.ascontiguousarray(v[b]),
            "costab": cos,
            "sintab": sin,
            "alibi": bias,
        })
    kw = {}
    if TRACE:
        kw = dict(trace=True)
    res = run_bass_kernel_spmd(nc, in_maps, list(range(B)), **kw)
    LAST_RESULTS = res
    out = np.stack(
        [np.ascontiguousarray(res.results[b]["ot"]).T for b in range(B)], axis=0
    )
    return out[None].astype(np.float32)



# revision 11
# speedup vs baseline: 1.2591x; 1.2591x over previous
"""RoPE + ALiBi single-head attention (B=8, T=2048, H=256) on 8 Trainium2
cores, batch-parallel (one batch element per core).

v2: all matmul operands in bf16 (host pre-casts inputs -- halves input DMA
and doubles DVE rope throughput), inputs spread over four DMA queues
(sync=k, scalar=cos/sin, gpsimd=q, tensor=v) so the serial-DMA preamble
shrinks, softmax denominator built as DVE quad-sums of the exp tiles plus
an all-ones [128,128] matmul that reduces AND broadcasts in one shot
(replaces 16 ones-matmuls/chunk on the PE and the gpsimd partition
broadcast), reciprocal via the single-op custom-DVE fast approx.

Per-core algorithm (all compute on device):
  qeT/keT = RoPE(qT/kT)                     [DVE bf16, 6 ops per 512-col
                                             chunk per tensor]
  scoresT[s,t] = sum_d keT[d,s]*qeT[d,t]    [PE bf16, 2 k-tiles -> PSUM f32]
  at[s,t] = exp(scoresT*scale + slope*s)    [ACT, PSUM->SBUF bf16]
     (the -slope*t alibi term is constant per softmax column and cancels)
  acc[q] = sum of 4 at tiles                [DVE bf16 quad trees]
  den_b[p,t] = sum_q ones128 @ acc[q]       [PE: 4 accumulating matmuls into
                                             a [128,512] PSUM bank -- the
                                             all-ones stationary makes every
                                             partition hold den[t]]
  recipb = 1/den_b                          [DVE custom fast reciprocal]
  outT[h,t] = (sum_s v[s,h]*at[s,t])*recipb [PE bf16 -> PSUM f32; DVE mul]
Host only reshapes/transposes/casts and precomputes rope/alibi tables.
"""
import math
from contextlib import ExitStack

import numpy as np
import ml_dtypes

import concourse.bacc as bacc
import concourse.tile as tile
from concourse import mybir
from concourse.bass_utils import run_bass_kernel_spmd

B, T, H = 8, 2048, 256
HALF = H // 2          # 128 (rope half, also partition dim)
NCHUNK = 4
CHUNK = T // NCHUNK    # 512 query columns per chunk
NS = T // 128          # 16 key tiles
ROPE_BASE = 10000.0
SLOPE = 2.0 ** (-8.0)
SCALE = 1.0 / math.sqrt(H)

F32 = mybir.dt.float32
BF16 = mybir.dt.bfloat16
EXP = mybir.ActivationFunctionType.Exp

TRACE = False           # test harness sets True for NTFF profiling
LAST_RESULTS = None     # BassKernelResults of the last run (for profiling)

_NC_CACHE = {}


def _build_nc():
    # qp/kp/cs are chunk-packed [128, 4096]: block c holds this chunk's two
    # operand halves side by side (1024 contiguous cols -> 2KB DMA rows, and
    # only one DMA per chunk so the framework's small DMA-semaphore pool
    # never wraps -- v2 lost ~12us to semaphore-reuse stalls)
    nc = bacc.Bacc("TRN2", target_bir_lowering=False, debug=False)
    qp_d = nc.dram_tensor("qp", [128, 2 * T], BF16, kind="ExternalInput").ap()
    kp_d = nc.dram_tensor("kp", [128, 2 * T], BF16, kind="ExternalInput").ap()
    vr_d = nc.dram_tensor("vr", [128, NS * H], BF16, kind="ExternalInput").ap()
    cs_d = nc.dram_tensor("cs", [128, 2 * T], BF16, kind="ExternalInput").ap()
    bias_d = nc.dram_tensor("alibi", [128, NS], F32, kind="ExternalInput").ap()
    ot_d = nc.dram_tensor("ot", [H, T], F32, kind="ExternalOutput").ap()

    with tile.TileContext(nc) as tc, ExitStack() as ctx:
        const = ctx.enter_context(tc.tile_pool(name="const", bufs=1))
        rpool = ctx.enter_context(tc.tile_pool(name="ropeout", bufs=1))
        vpool = ctx.enter_context(tc.tile_pool(name="vpool", bufs=1))
        stage = ctx.enter_context(tc.tile_pool(name="stage", bufs=1))
        atp = ctx.enter_context(tc.tile_pool(name="atp", bufs=20))
        accp = ctx.enter_context(tc.tile_pool(name="accp", bufs=6))
        dn = ctx.enter_context(tc.tile_pool(name="dn", bufs=2))
        onp = ctx.enter_context(tc.tile_pool(name="onp", bufs=4))
        ps1p = ctx.enter_context(tc.tile_pool(name="ps1", bufs=3, space="PSUM"))
        ps2p = ctx.enter_context(tc.tile_pool(name="ps2", bufs=3, space="PSUM"))
        pdnp = ctx.enter_context(tc.tile_pool(name="pdn", bufs=2, space="PSUM"))

        # constants: alibi bias rows, all-ones [128,128] stationary for the
        # reduce+broadcast denominator matmul
        biasb = const.tile([128, NS], F32)
        nc.gpsimd.dma_start(biasb[:], bias_d[:])
        ones_f = const.tile([128, 128], F32)
        nc.vector.memset(ones_f[:], 1.0)
        ones_b = const.tile([128, 128], BF16)
        nc.vector.tensor_copy(ones_b[:], ones_f[:])

        # persistent bf16 operands for the two GEMMs
        qe = [rpool.tile([128, T], BF16, name=f"qe{i}", tag=f"qe{i}")
              for i in range(2)]
        ke = [rpool.tile([128, T], BF16, name=f"ke{i}", tag=f"ke{i}")
              for i in range(2)]
        vr = vpool.tile([128, NS * H], BF16)

        # chunk-packed staging tiles (layout mirrors the dram tensors)
        csb = stage.tile([128, 2 * T], BF16, tag="csb")
        ksb = stage.tile([128, 2 * T], BF16, tag="ksb")
        qsb = stage.tile([128, 2 * T], BF16, tag="qsb")

        # input DMAs: one per chunk per tensor on three queues (sync=k then
        # half of v, scalar=cos/sin then half of v, gpsimd=q; v must beat
        # the first GEMM2, ~18us in)
        for cc in range(NCHUNK):
            col = slice(cc * 2 * CHUNK, (cc + 1) * 2 * CHUNK)
            nc.sync.dma_start(ksb[:, col], kp_d[:, col])
            nc.scalar.dma_start(csb[:, col], cs_d[:, col])
            nc.gpsimd.dma_start(qsb[:, col], qp_d[:, col])
        half = NS * H // 2
        nc.sync.dma_start(vr[:, 0:half], vr_d[:, 0:half])
        nc.scalar.dma_start(vr[:, half:], vr_d[:, half:])

        # PE warm-up: the HAM clock gate needs ~3.4us of sustained activity
        # before it passes the full 2.4 GHz clock; burn dummy matmuls while
        # the DMAs + first ropes run so the real stream starts warm
        pdum = pdnp.tile([128, CHUNK], F32, tag="pden", name="pdum")
        for _ in range(40):
            nc.tensor.matmul(pdum[:, 0:128], ones_b[:], ones_b[:],
                             start=True, stop=True)

        def rope(cc, src, dst, tmptag):
            """packed src block cc -> dst halves at columns [512cc, 512cc+512)
            dst0 = s0*cos - s1*sin ; dst1 = s1*cos + s0*sin"""
            col = slice(cc * CHUNK, (cc + 1) * CHUNK)
            s0 = src[:, 2 * cc * CHUNK:(2 * cc + 1) * CHUNK]
            s1 = src[:, (2 * cc + 1) * CHUNK:(2 * cc + 2) * CHUNK]
            co = csb[:, 2 * cc * CHUNK:(2 * cc + 1) * CHUNK]
            si = csb[:, (2 * cc + 1) * CHUNK:(2 * cc + 2) * CHUNK]
            nc.vector.tensor_mul(dst[0][:, col], s0, co)
            tmp = stage.tile([128, CHUNK], BF16, tag="rtmp", bufs=3,
                             name=f"tmp{tmptag}")
            nc.vector.tensor_mul(tmp[:], s1, si)
            nc.vector.tensor_sub(dst[0][:, col], dst[0][:, col], tmp[:])
            nc.vector.tensor_mul(dst[1][:, col], s1, co)
            tmp2 = stage.tile([128, CHUNK], BF16, tag="rtmp", bufs=3,
                              name=f"tmp2{tmptag}")
            nc.vector.tensor_mul(tmp2[:], s0, si)
            nc.vector.tensor_add(dst[1][:, col], dst[1][:, col], tmp2[:])

        # DVE order: q chunk0 + k chunk0 first (unblocks GEMM1), then the
        # remaining k chunks (GEMM1 s-tiles 4..15 of chunk 0 need them)
        rope(0, qsb, qe, "q0")
        rope(0, ksb, ke, "k0")
        for cc in range(1, NCHUNK):
            rope(cc, ksb, ke, f"k{cc}")

        mm = nc.tensor.matmul
        for c in range(NCHUNK):
            tcol = slice(c * CHUNK, (c + 1) * CHUNK)
            at_tiles = []
            for s in range(NS):
                p1 = ps1p.tile([128, CHUNK], F32)
                mm(p1[:], ke[0][:, s * 128:(s + 1) * 128], qe[0][:, tcol],
                   start=True, stop=False)
                mm(p1[:], ke[1][:, s * 128:(s + 1) * 128], qe[1][:, tcol],
                   start=False, stop=True)
                at = atp.tile([128, CHUNK], BF16, tag="at")
                nc.scalar.activation(at[:], p1[:], EXP,
                                     bias=biasb[:, s:s + 1], scale=SCALE)
                at_tiles.append(at)
            if c + 1 < NCHUNK:
                # rope next chunk's q columns ahead of its GEMM1
                rope(c + 1, qsb, qe, f"q{c + 1}")

            # denominator: DVE quad-sums of the 16 at tiles, then 4
            # accumulating all-ones matmuls -> [128,512] PSUM where every
            # partition carries den[t] (reduce + broadcast in one)
            accs = []
            for qd in range(4):
                acc = accp.tile([128, CHUNK], BF16, tag="acc",
                                name=f"acc{c}_{qd}")
                nc.vector.tensor_add(acc[:], at_tiles[4 * qd][:],
                                     at_tiles[4 * qd + 1][:])
                nc.vector.tensor_add(acc[:], acc[:], at_tiles[4 * qd + 2][:])
                nc.vector.tensor_add(acc[:], acc[:], at_tiles[4 * qd + 3][:])
                accs.append(acc)

            # PE stream: GEMM2 (both h-tiles) first, then the 4 den matmuls
            # (their DVE quad inputs are only ready near the chunk's end)
            p2s = []
            for h in range(2):
                p2 = ps2p.tile([128, CHUNK], F32)
                for s in range(NS):
                    mm(p2[:], vr[:, s * H + h * 128: s * H + h * 128 + 128],
                       at_tiles[s][:], start=(s == 0), stop=(s == NS - 1))
                p2s.append(p2)
            pden = pdnp.tile([128, CHUNK], F32, tag="pden", name=f"pden{c}")
            for qd in range(4):
                mm(pden[:], ones_b[:], accs[qd][:],
                   start=(qd == 0), stop=(qd == 3))

            recipb = dn.tile([128, CHUNK], F32, tag="recipb")
            nc.vector.reciprocal_approx_fast(recipb[:], pden[:])
            for h in range(2):
                on = onp.tile([128, CHUNK], F32)
                nc.vector.tensor_mul(on[:], p2s[h][:], recipb[:])
                nc.sync.dma_start(ot_d[h * 128:(h + 1) * 128, tcol], on[:])

    nc.compile()
    return nc


def _get_nc():
    if "nc" not in _NC_CACHE:
        _NC_CACHE["nc"] = _build_nc()
    return _NC_CACHE["nc"]


def _tables():
    j = np.arange(HALF, dtype=np.float64)
    inv = ROPE_BASE ** (-2.0 * j / H)
    t = np.arange(T, dtype=np.float64)
    fr = np.outer(inv, t)                       # [128, T]
    cos = np.cos(fr).astype(ml_dtypes.bfloat16)
    sin = np.sin(fr).astype(ml_dtypes.bfloat16)
    p = np.arange(128, dtype=np.float64)[:, None]
    sidx = p + 128.0 * np.arange(NS, dtype=np.float64)[None, :]
    bias = (SLOPE * sidx).astype(np.float32)    # [128, NS]
    return cos, sin, bias


def kernel(q, k, v):
    global LAST_RESULTS
    q = np.asarray(q, dtype=np.float32)
    k = np.asarray(k, dtype=np.float32)
    v = np.asarray(v, dtype=np.float32)
    assert q.shape == (B, T, H), q.shape

    nc = _get_nc()
    cos, sin, bias = _tables()

    def pack(h0, h1):
        # [128, 2T]: block c = [h0 cols 512c..512c+512 | h1 same cols]
        blocks = []
        for c in range(NCHUNK):
            col = slice(c * CHUNK, (c + 1) * CHUNK)
            blocks.append(h0[:, col])
            blocks.append(h1[:, col])
        return np.ascontiguousarray(np.concatenate(blocks, axis=1))

    cs = pack(cos, sin)
    in_maps = []
    for b in range(B):
        vrb = np.concatenate(
            [v[b, s * 128:(s + 1) * 128, :] for s in range(NS)], axis=1
        ).astype(ml_dtypes.bfloat16)            # [128, NS*H]
        qt = q[b].T.astype(ml_dtypes.bfloat16)
        kt = k[b].T.astype(ml_dtypes.bfloat16)
        in_maps.append({
            "qp": pack(qt[0:128], qt[128:256]),
            "kp": pack(kt[0:128], kt[128:256]),
            "vr": vrb,
            "cs": cs,
            "alibi": bias,
        })
    kw = {}
    if TRACE:
        kw = dict(trace=True)
    res = run_bass_kernel_spmd(nc, in_maps, list(range(B)), **kw)
    LAST_RESULTS = res
    out = np.stack(
        [np.ascontiguousarray(res.results[b]["ot"]).T for b in range(B)], axis=0
    )
    return out[None].astype(np.float32)


# revision 17
# speedup vs baseline: 1.2900x; 1.0246x over previous
"""RoPE + ALiBi single-head attention (B=8, T=2048, H=256) on 8 Trainium2
cores, batch-parallel (one batch element per core).

v2: all matmul operands in bf16 (host pre-casts inputs -- halves input DMA
and doubles DVE rope throughput), inputs spread over four DMA queues
(sync=k, scalar=cos/sin, gpsimd=q, tensor=v) so the serial-DMA preamble
shrinks, softmax denominator built as DVE quad-sums of the exp tiles plus
an all-ones [128,128] matmul that reduces AND broadcasts in one shot
(replaces 16 ones-matmuls/chunk on the PE and the gpsimd partition
broadcast), reciprocal via the single-op custom-DVE fast approx.

Per-core algorithm (all compute on device):
  qeT/keT = RoPE(qT/kT)                     [DVE bf16, 6 ops per 512-col
                                             chunk per tensor]
  scoresT[s,t] = sum_d keT[d,s]*qeT[d,t]    [PE bf16, 2 k-tiles -> PSUM f32]
  at[s,t] = exp(scoresT*scale + slope*s)    [ACT, PSUM->SBUF bf16]
     (the -slope*t alibi term is constant per softmax column and cancels)
  acc[q] = sum of 4 at tiles                [DVE bf16 quad trees]
  den_b[p,t] = sum_q ones128 @ acc[q]       [PE: 4 accumulating matmuls into
                                             a [128,512] PSUM bank -- the
                                             all-ones stationary makes every
                                             partition hold den[t]]
  recipb = 1/den_b                          [DVE custom fast reciprocal]
  outT[h,t] = (sum_s v[s,h]*at[s,t])*recipb [PE bf16 -> PSUM f32; DVE mul]
Host only reshapes/transposes/casts and precomputes rope/alibi tables.
"""
import math
from contextlib import ExitStack

import numpy as np
import ml_dtypes

import concourse.bacc as bacc
import concourse.tile as tile
from concourse import mybir
from concourse.bass_utils import run_bass_kernel_spmd

B, T, H = 8, 2048, 256
HALF = H // 2          # 128 (rope half, also partition dim)
NCHUNK = 4
CHUNK = T // NCHUNK    # 512 query columns per chunk
NS = T // 128          # 16 key tiles
ROPE_BASE = 10000.0
SLOPE = 2.0 ** (-8.0)
SCALE = 1.0 / math.sqrt(H)

F32 = mybir.dt.float32
BF16 = mybir.dt.bfloat16
EXP = mybir.ActivationFunctionType.Exp

TRACE = False           # test harness sets True for NTFF profiling
LAST_RESULTS = None     # BassKernelResults of the last run (for profiling)

_NC_CACHE = {}


def _build_nc():
    # qp/kp/cs are chunk-packed [128, 4096]: block c holds this chunk's two
    # operand halves side by side (1024 contiguous cols -> 2KB DMA rows, and
    # only one DMA per chunk so the framework's small DMA-semaphore pool
    # never wraps -- v2 lost ~12us to semaphore-reuse stalls)
    nc = bacc.Bacc("TRN2", target_bir_lowering=False, debug=False)
    qp_d = nc.dram_tensor("qp", [128, 2 * T], BF16, kind="ExternalInput").ap()
    kp_d = nc.dram_tensor("kp", [128, 2 * T], BF16, kind="ExternalInput").ap()
    vr_d = nc.dram_tensor("vr", [128, NS * H], BF16, kind="ExternalInput").ap()
    cs_d = nc.dram_tensor("cs", [128, 2 * T], BF16, kind="ExternalInput").ap()
    bias_d = nc.dram_tensor("alibi", [128, NS], F32, kind="ExternalInput").ap()
    ot_d = nc.dram_tensor("ot", [H, T], BF16, kind="ExternalOutput").ap()

    with tile.TileContext(nc) as tc, ExitStack() as ctx:
        const = ctx.enter_context(tc.tile_pool(name="const", bufs=1))
        rpool = ctx.enter_context(tc.tile_pool(name="ropeout", bufs=1))
        vpool = ctx.enter_context(tc.tile_pool(name="vpool", bufs=1))
        stage = ctx.enter_context(tc.tile_pool(name="stage", bufs=1))
        atp = ctx.enter_context(tc.tile_pool(name="atp", bufs=20))
        accp = ctx.enter_context(tc.tile_pool(name="accp", bufs=6))
        dn = ctx.enter_context(tc.tile_pool(name="dn", bufs=2))
        onp = ctx.enter_context(tc.tile_pool(name="onp", bufs=4))
        ps1p = ctx.enter_context(tc.tile_pool(name="ps1", bufs=3, space="PSUM"))
        ps2p = ctx.enter_context(tc.tile_pool(name="ps2", bufs=3, space="PSUM"))
        pdnp = ctx.enter_context(tc.tile_pool(name="pdn", bufs=2, space="PSUM"))

        # constants: alibi bias rows, all-ones [128,128] stationary for the
        # reduce+broadcast denominator matmul
        biasb = const.tile([128, NS], F32)
        ones_f = const.tile([128, 128], F32)
        nc.vector.memset(ones_f[:], 1.0)
        ones_b = const.tile([128, 128], BF16)
        nc.vector.tensor_copy(ones_b[:], ones_f[:])

        # persistent bf16 operands for the two GEMMs
        qe = [rpool.tile([128, T], BF16, name=f"qe{i}", tag=f"qe{i}")
              for i in range(2)]
        ke = [rpool.tile([128, T], BF16, name=f"ke{i}", tag=f"ke{i}")
              for i in range(2)]
        vr = vpool.tile([128, NS * H], BF16)

        # chunk-packed staging tiles (layout mirrors the dram tensors)
        csb = stage.tile([128, 2 * T], BF16, tag="csb")
        ksb = stage.tile([128, 2 * T], BF16, tag="ksb")
        qsb = stage.tile([128, 2 * T], BF16, tag="qsb")

        # input DMAs: each queue sustains only ~110-130 GB/s (row packets
        # striped over 16 DMA engines), so order by need: chunk-0 operands
        # first on every queue, v halves last (first GEMM2 is ~21us in).
        # scalar's queue carries only cos/sin: its triggers sit in the ACT
        # instruction stream and must never block the exp chain.
        nc.gpsimd.dma_start(biasb[:], bias_d[:])
        for cc in range(NCHUNK):
            col = slice(cc * 2 * CHUNK, (cc + 1) * 2 * CHUNK)
            nc.sync.dma_start(ksb[:, col], kp_d[:, col])
            nc.scalar.dma_start(csb[:, col], cs_d[:, col])
            nc.gpsimd.dma_start(qsb[:, col], qp_d[:, col])
        half = NS * H // 2
        nc.sync.dma_start(vr[:, 0:half], vr_d[:, 0:half])
        nc.gpsimd.dma_start(vr[:, half:], vr_d[:, half:])

        # PE warm-up: the HAM clock gate needs ~3.4us of sustained activity
        # before it passes the full 2.4 GHz clock; burn dummy matmuls while
        # the DMAs + first ropes run so the real stream starts warm
        pdum = pdnp.tile([128, CHUNK], F32, tag="pden", name="pdum")
        for _ in range(56):
            nc.tensor.matmul(pdum[:, 0:128], ones_b[:], ones_b[:],
                             start=True, stop=True)

        def rope(cc, src, dst, tmptag):
            """packed src block cc -> dst halves at columns [512cc, 512cc+512)
            dst0 = s0*cos - s1*sin ; dst1 = s1*cos + s0*sin"""
            col = slice(cc * CHUNK, (cc + 1) * CHUNK)
            s0 = src[:, 2 * cc * CHUNK:(2 * cc + 1) * CHUNK]
            s1 = src[:, (2 * cc + 1) * CHUNK:(2 * cc + 2) * CHUNK]
            co = csb[:, 2 * cc * CHUNK:(2 * cc + 1) * CHUNK]
            si = csb[:, (2 * cc + 1) * CHUNK:(2 * cc + 2) * CHUNK]
            nc.vector.tensor_mul(dst[0][:, col], s0, co)
            tmp = stage.tile([128, CHUNK], BF16, tag="rtmp", bufs=3,
                             name=f"tmp{tmptag}")
            nc.vector.tensor_mul(tmp[:], s1, si)
            nc.vector.tensor_sub(dst[0][:, col], dst[0][:, col], tmp[:])
            nc.vector.tensor_mul(dst[1][:, col], s1, co)
            tmp2 = stage.tile([128, CHUNK], BF16, tag="rtmp", bufs=3,
                              name=f"tmp2{tmptag}")
            nc.vector.tensor_mul(tmp2[:], s0, si)
            nc.vector.tensor_add(dst[1][:, col], dst[1][:, col], tmp2[:])

        # DVE order tuned so chunk 0's GEMM1 never stalls: k0, k1, then q0
        # (first matmul fires right after), then k2/k3 land just ahead of
        # the s-tiles that need them
        rope(0, ksb, ke, "k0")
        rope(1, ksb, ke, "k1")
        rope(0, qsb, qe, "q0")
        rope(2, ksb, ke, "k2")
        rope(3, ksb, ke, "k3")

        mm = nc.tensor.matmul
        for c in range(NCHUNK):
            tcol = slice(c * CHUNK, (c + 1) * CHUNK)
            at_tiles = []
            for s in range(NS):
                p1 = ps1p.tile([128, CHUNK], F32)
                mm(p1[:], ke[0][:, s * 128:(s + 1) * 128], qe[0][:, tcol],
                   start=True, stop=False)
                mm(p1[:], ke[1][:, s * 128:(s + 1) * 128], qe[1][:, tcol],
                   start=False, stop=True)
                at = atp.tile([128, CHUNK], BF16, tag="at")
                nc.scalar.activation(at[:], p1[:], EXP,
                                     bias=biasb[:, s:s + 1], scale=SCALE)
                at_tiles.append(at)
            if c + 1 < NCHUNK:
                # rope next chunk's q columns ahead of its GEMM1
                rope(c + 1, qsb, qe, f"q{c + 1}")

            # denominator: DVE quad-sums of the 16 at tiles, then 4
            # accumulating all-ones matmuls -> [128,512] PSUM where every
            # partition carries den[t] (reduce + broadcast in one)
            accs = []
            for qd in range(4):
                acc = accp.tile([128, CHUNK], BF16, tag="acc",
                                name=f"acc{c}_{qd}")
                nc.vector.tensor_add(acc[:], at_tiles[4 * qd][:],
                                     at_tiles[4 * qd + 1][:])
                nc.vector.tensor_add(acc[:], acc[:], at_tiles[4 * qd + 2][:])
                nc.vector.tensor_add(acc[:], acc[:], at_tiles[4 * qd + 3][:])
                accs.append(acc)

            # PE stream: GEMM2 (both h-tiles) first, then the 4 den matmuls
            # (their DVE quad inputs are only ready near the chunk's end)
            p2s = []
            for h in range(2):
                p2 = ps2p.tile([128, CHUNK], F32)
                for s in range(NS):
                    mm(p2[:], vr[:, s * H + h * 128: s * H + h * 128 + 128],
                       at_tiles[s][:], start=(s == 0), stop=(s == NS - 1))
                p2s.append(p2)
            pden = pdnp.tile([128, CHUNK], F32, tag="pden", name=f"pden{c}")
            for qd in range(4):
                mm(pden[:], ones_b[:], accs[qd][:],
                   start=(qd == 0), stop=(qd == 3))

            recipb = dn.tile([128, CHUNK], F32, tag="recipb")
            nc.vector.reciprocal_approx_fast(recipb[:], pden[:])
            for h in range(2):
                on = onp.tile([128, CHUNK], BF16)
                nc.vector.tensor_mul(on[:], p2s[h][:], recipb[:])
                eng = nc.sync if h == 0 else nc.gpsimd
                eng.dma_start(ot_d[h * 128:(h + 1) * 128, tcol], on[:])

    nc.compile()
    return nc


def _get_nc():
    if "nc" not in _NC_CACHE:
        _NC_CACHE["nc"] = _build_nc()
    return _NC_CACHE["nc"]


def _tables():
    j = np.arange(HALF, dtype=np.float64)
    inv = ROPE_BASE ** (-2.0 * j / H)
    t = np.arange(T, dtype=np.float64)
    fr = np.outer(inv, t)                       # [128, T]
    cos = np.cos(fr).astype(ml_dtypes.bfloat16)
    sin = np.sin(fr).astype(ml_dtypes.bfloat16)
    p = np.arange(128, dtype=np.float64)[:, None]
    sidx = p + 128.0 * np.arange(NS, dtype=np.float64)[None, :]
    bias = (SLOPE * sidx).astype(np.float32)    # [128, NS]
    return cos, sin, bias


def kernel(q, k, v):
    global LAST_RESULTS
    q = np.asarray(q, dtype=np.float32)
    k = np.asarray(k, dtype=np.float32)
    v = np.asarray(v, dtype=np.float32)
    assert q.shape == (B, T, H), q.shape

    nc = _get_nc()
    cos, sin, bias = _tables()

    def pack(h0, h1):
        # [128, 2T]: block c = [h0 cols 512c..512c+512 | h1 same cols]
        blocks = []
        for c in range(NCHUNK):
            col = slice(c * CHUNK, (c + 1) * CHUNK)
            blocks.append(h0[:, col])
            blocks.append(h1[:, col])
        return np.ascontiguousarray(np.concatenate(blocks, axis=1))

    cs = pack(cos, sin)
    in_maps = []
    for b in range(B):
        vrb = np.concatenate(
            [v[b, s * 128:(s + 1) * 128, :] for s in range(NS)], axis=1
        ).astype(ml_dtypes.bfloat16)            # [128, NS*H]
        qt = q[b].T.astype(ml_dtypes.bfloat16)
        kt = k[b].T.astype(ml_dtypes.bfloat16)
        in_maps.append({
            "qp": pack(qt[0:128], qt[128:256]),
            "kp": pack(kt[0:128], kt[128:256]),
            "vr": vrb,
            "cs": cs,
            "alibi": bias,
        })
    kw = {}
    if TRACE:
        kw = dict(trace=True)
    res = run_bass_kernel_spmd(nc, in_maps, list(range(B)), **kw)
    LAST_RESULTS = res
    out = np.stack(
        [np.asarray(res.results[b]["ot"]).astype(np.float32).T for b in range(B)],
        axis=0,
    )
    return np.ascontiguousarray(out[None])
